# revision 12
# baseline (speedup 1.0000x reference)
"""TRN2 Bass kernel: masked multi-head attention block (B=4, S=2048, C=768, H=12).

Sharding: 8 cores = 4 batches x 2 head-groups (6 heads each).  Each core runs a
flash-attention-style Bass/Tile kernel over its (batch, head-group) shard:

  qT/kT: [384, S] feature-major projections from xT (q pre-scaled by hd^-0.5)
  v:     [S, 6*65] natural layout, a ones column appended per head
  scoresT[k, q] = k . q (contract hd=64, head pairs row-packed on the PE array)
  mask folded in as -1e5*maskT via identity-matmul on PE / in-place DVE add
  pT = exp(scoresT) on the scalar engine (psum -> sbuf)
  avT[65, 512] accumulated over key chunks; row 64 = softmax denominator (ones col)
  attn_outT = avT[0:64] * recip(denominator)  (partition-broadcast on gpsimd)
  y_partial = attn_outT.T @ w_projT slice  (row-parallel output projection)

Host-side: transposes/slices the weights per core, converts the mask to
pre-scaled bf16, sums the two per-batch partials, and adds b_proj.
"""

from contextlib import ExitStack

import numpy as np

import concourse.tile as tile
from concourse import bacc, mybir
from concourse.bass_utils import run_bass_kernel_spmd

F32 = mybir.dt.float32
F32R = mybir.dt.float32r
BF16 = mybir.dt.bfloat16

MASK_NEG = -100000.0
B, S, C, H = 4, 2048, 768, 12
HD = 64
H_PER_CORE = 6
D_CORE = H_PER_CORE * HD  # 384
QBLK = 512
GRP = 2
N_CORES = 8
MASK_DVE_FRAC = 0.85


def _r(ap):
    return ap.bitcast(F32R)


def _build_kernel(mask_dve_frac=MASK_DVE_FRAC):
    nc = bacc.Bacc(
        trn_type="TRN2", target_bir_lowering=False, debug=False, num_devices=N_CORES
    )
    KC = S // 128
    QB = S // QBLK
    NB = S // QBLK
    ST = S // 128
    groups = []
    for half in range(KC // 8):
        base = half * 8
        for g0, gs in ((0, 2), (2, 2), (4, 2), (6, 2)):
            groups.append((base + g0, gs))

    xT = nc.dram_tensor("xT", [C, S], F32R, kind="ExternalInput").ap()
    wq = nc.dram_tensor("wq", [C, D_CORE], F32R, kind="ExternalInput").ap()
    wk = nc.dram_tensor("wk", [C, D_CORE], F32R, kind="ExternalInput").ap()
    wv = nc.dram_tensor("wv", [C, D_CORE], F32R, kind="ExternalInput").ap()
    wproj = nc.dram_tensor("wproj", [D_CORE, C], F32R, kind="ExternalInput").ap()
    vones = nc.dram_tensor("vones", [128, S // 128 * H_PER_CORE], F32R, kind="ExternalInput").ap()
    maskT = nc.dram_tensor("maskT", [S, S], BF16, kind="ExternalInput").ap()
    ident = nc.dram_tensor("ident", [128, 128], BF16, kind="ExternalInput").ap()
    y = nc.dram_tensor("y", [S, C], F32, kind="ExternalOutput").ap()

    with tile.TileContext(nc) as tc, ExitStack() as ctx:
        consts = ctx.enter_context(tc.tile_pool(name="consts", bufs=1))
        qkv_pool = ctx.enter_context(tc.tile_pool(name="qkv", bufs=1))

        ident_sb = consts.tile([128, 128], BF16)
        nc.sync.dma_start(ident_sb[:], ident[:])
        wproj_sb = consts.tile([128, 3, C], F32R)
        nc.sync.dma_start(wproj_sb[:], wproj.rearrange("(t p) o -> p t o", p=128))

        qT_sb = qkv_pool.tile([128, 3, S], F32R)
        kT_sb = qkv_pool.tile([128, 3, S], F32R)
        vaug_sb = qkv_pool.tile([128, ST, H_PER_CORE * (HD + 1)], F32R)
        attn_sb = qkv_pool.tile([128, 3, S], F32R)

        # ones columns (softmax denominator) come from DRAM
        vaug_ones = vaug_sb.rearrange("p st (h u) -> p st h u", u=HD + 1)[:, :, :, HD]
        nc.sync.dma_start(
            vaug_ones, vones.rearrange("p (st h) -> p st h", h=H_PER_CORE)
        )

        # ---------------- phase 1: qkv projections ----------------
        with ExitStack() as p1:
            wpool = p1.enter_context(tc.tile_pool(name="w1", bufs=1))
            xpool = p1.enter_context(tc.tile_pool(name="x1", bufs=1))
            ps1 = p1.enter_context(tc.tile_pool(name="ps1", bufs=3, space="PSUM"))
            psv1 = p1.enter_context(tc.tile_pool(name="psv1", bufs=2, space="PSUM"))

            wq_sb = wpool.tile([128, 6, D_CORE], F32R)
            wk_sb = wpool.tile([128, 6, D_CORE], F32R)
            wv_sb = wpool.tile([128, 6, D_CORE], F32R)
            for w_ap, w_sb in ((wq, wq_sb), (wk, wk_sb), (wv, wv_sb)):
                nc.sync.dma_start(
                    w_sb[:], w_ap.rearrange("(t p) d -> p t d", p=128)
                )
            xT_sb = xpool.tile([128, 6, S], F32R)
            nc.sync.dma_start(xT_sb[:], xT.rearrange("(t p) s -> p t s", p=128))

            cp_i = 0
            for w_sb, dst in ((wq_sb, qT_sb), (wk_sb, kT_sb)):
                for m in range(3):
                    for nb in range(NB):
                        ps = ps1.tile([128, QBLK], F32, tag="psqk", name="psqk")
                        for k in range(6):
                            nc.tensor.matmul(
                                ps[:],
                                w_sb[:, k, m * 128 : (m + 1) * 128],
                                xT_sb[:, k, nb * QBLK : (nb + 1) * QBLK],
                                start=(k == 0),
                                stop=(k == 5),
                            )
                        dst_ap = dst[:, m, nb * QBLK : (nb + 1) * QBLK]
                        if cp_i % 2 == 0:
                            nc.vector.tensor_copy(dst_ap, ps[:])
                        else:
                            nc.scalar.copy(dst_ap, ps[:])
                        cp_i += 1

            for st in range(ST):
                psv = psv1.tile([128, D_CORE], F32, tag="psv", name="psv")
                for k in range(6):
                    nc.tensor.matmul(
                        psv[:],
                        xT_sb[:, k, st * 128 : (st + 1) * 128],
                        wv_sb[:, k, :],
                        start=(k == 0),
                        stop=(k == 5),
                    )
                for h in range(H_PER_CORE):
                    dst = vaug_sb[:, st, h * (HD + 1) : h * (HD + 1) + HD]
                    src = psv[:, h * HD : (h + 1) * HD]
                    if h % 2 == 0:
                        nc.vector.tensor_copy(dst, src)
                    else:
                        nc.scalar.copy(dst, src)

        # ---------------- phase 2: attention ----------------
        with ExitStack() as p2:
            mpool = p2.enter_context(tc.tile_pool(name="mask", bufs=4))
            ppool = p2.enter_context(tc.tile_pool(name="pT", bufs=5))
            dpool = p2.enter_context(tc.tile_pool(name="div", bufs=2))
            bpool = p2.enter_context(tc.tile_pool(name="bcast", bufs=2))
            apool = p2.enter_context(tc.tile_pool(name="avsb", bufs=1))
            ps_s = p2.enter_context(tc.tile_pool(name="ps_s", bufs=3, space="PSUM"))
            ps_av = p2.enter_context(tc.tile_pool(name="ps_av", bufs=2, space="PSUM"))

            mask_idx = 0
            maskT_r = maskT.rearrange("(kc p) q -> p kc q", p=128)
            mask_cache = {}

            def load_mask(qb_i):
                halves = []
                for half in range(KC // 8):
                    mh = mpool.tile([128, 8, QBLK], BF16, tag="mask", name="mask_h")
                    nc.sync.dma_start(
                        mh[:],
                        maskT_r[
                            :,
                            half * 8 : (half + 1) * 8,
                            qb_i * QBLK : (qb_i + 1) * QBLK,
                        ],
                    )
                    halves.append(mh)
                return halves

            mask_cache[0] = load_mask(0)
            for qb in range(QB):
                if qb + 1 < QB:
                    mask_cache[qb + 1] = load_mask(qb + 1)
                mask_halves = mask_cache.pop(qb)
                dstack = dpool.tile([H_PER_CORE, QBLK], F32, tag="dstack", name="dstack")
                recip = dpool.tile([H_PER_CORE, QBLK], F32, tag="recip", name="recip")
                av_all = apool.tile([HD + 1, H_PER_CORE, QBLK], F32, tag="av_all", name="av_all")
                av_keep = []

                for hp in range(3):
                    hA, hB = 2 * hp, 2 * hp + 1
                    av = [
                        ps_av.tile([HD + 1, QBLK], F32, tag="av", name=f"av{hp}a"),
                        ps_av.tile([HD + 1, QBLK], F32, tag="av", name=f"av{hp}b"),
                    ]
                    for (g0, gs) in groups:
                        mh = mask_halves[g0 // 8]
                        moff = g0 % 8
                        sc = [
                            ps_s.tile([128, GRP, QBLK], F32, tag="sc", name="scA"),
                            ps_s.tile([128, GRP, QBLK], F32, tag="sc", name="scB"),
                        ]
                        dve_heads = []
                        for i, h in ((0, hA), (1, hB)):
                            on_dve = (mask_idx % 10) < int(round(mask_dve_frac * 10))
                            mask_idx += 1
                            if on_dve:
                                dve_heads.append(i)
                            else:
                                for c in range(gs):
                                    nc.tensor.matmul(
                                        sc[i][:, c, :],
                                        ident_sb[:],
                                        mh[:, moff + c, :],
                                        start=True,
                                        stop=False,
                                    )
                            row0 = (h % 2) * HD
                            for c in range(gs):
                                kc = g0 + c
                                nc.tensor.matmul(
                                    sc[i][:, c, :],
                                    kT_sb[
                                        row0 : row0 + HD,
                                        h // 2,
                                        kc * 128 : (kc + 1) * 128,
                                    ],
                                    qT_sb[
                                        row0 : row0 + HD,
                                        h // 2,
                                        qb * QBLK : (qb + 1) * QBLK,
                                    ],
                                    start=on_dve,
                                    stop=True,
                                    tile_position=(row0, 0),
                                )
                        for i in dve_heads:
                            nc.vector.tensor_add(
                                sc[i][:, :gs, :],
                                sc[i][:, :gs, :],
                                mh[:, moff : moff + gs, :],
                            )

                        for i, h in ((0, hA), (1, hB)):
                            pT = ppool.tile([128, GRP, QBLK], F32R, tag="pT", name="pT")
                            nc.scalar.activation(
                                pT[:, :gs, :],
                                sc[i][:, :gs, :],
                                mybir.ActivationFunctionType.Exp,
                            )
                            for c in range(gs):
                                kc = g0 + c
                                nc.tensor.matmul(
                                    av[i][:],
                                    vaug_sb[:, kc, h * (HD + 1) : (h + 1) * (HD + 1)],
                                    _r(pT[:, c, :]),
                                    start=(kc == 0),
                                    stop=(kc == KC - 1),
                                )

                    for i, h in ((0, hA), (1, hB)):
                        nc.vector.tensor_copy(av_all[:, h, :], av[i][:])
                        av_keep.append(h)

                # gather all 6 denominator rows in one partition-shift DMA
                nc.gpsimd.dma_start(dstack[:], av_all[HD : HD + 1, :, :])
                nc.vector.reciprocal_approx_fast(recip[:], dstack[:])
                r6 = bpool.tile([1, H_PER_CORE, QBLK], F32, tag="r6", name="r6", bufs=1)
                nc.gpsimd.dma_start(r6[:], recip[:])
                tmp_all = bpool.tile(
                    [HD, 3, QBLK], F32R, tag="tmp_all", name="tmp_all", bufs=1
                )
                for h in av_keep:
                    bc = bpool.tile([HD, QBLK], F32, tag="bc", name="bc")
                    nc.gpsimd.partition_broadcast(bc[:], r6[:, h, :])
                    if h % 2 == 0:
                        dst = attn_sb[:HD, h // 2, qb * QBLK : (qb + 1) * QBLK]
                        nc.vector.tensor_mul(dst, av_all[:HD, h, :], bc[:])
                    else:
                        nc.vector.tensor_mul(
                            tmp_all[:, h // 2, :], av_all[:HD, h, :], bc[:]
                        )
                nc.gpsimd.dma_start(
                    attn_sb[HD:128, :, qb * QBLK : (qb + 1) * QBLK], tmp_all[:]
                )

        # ---------------- phase 3: output projection ----------------
        with ExitStack() as p3:
            ypool = p3.enter_context(tc.tile_pool(name="y", bufs=3))
            ps_y = p3.enter_context(tc.tile_pool(name="ps_y", bufs=4, space="PSUM"))
            y_r = y.rearrange("(st p) o -> st p o", p=128)
            for st in range(ST):
                y_sb = ypool.tile([128, C], F32, tag="ysb", name="y_sb")
                for nb2 in range(2):
                    ps = ps_y.tile([128, 384], F32, tag="psy", name="psy")
                    for k3 in range(3):
                        nc.tensor.matmul(
                            ps[:],
                            attn_sb[:, k3, st * 128 : (st + 1) * 128],
                            wproj_sb[:, k3, nb2 * 384 : (nb2 + 1) * 384],
                            start=(k3 == 0),
                            stop=(k3 == 2),
                        )
                    if nb2 == 0:
                        nc.vector.tensor_copy(y_sb[:, :384], ps[:])
                    else:
                        nc.scalar.copy(y_sb[:, 384:], ps[:])
                nc.sync.dma_start(y_r[st], y_sb[:])

    nc.compile()
    return nc


def _prep_core_inputs(x, mask, w_qkv, w_proj, core):
    import ml_dtypes

    b, g = core // 2, core % 2
    scale = HD ** -0.5
    s0, s1 = 384 * g, 384 * (g + 1)
    return {
        "xT": np.ascontiguousarray(x[b].T),
        "wq": np.ascontiguousarray((w_qkv[s0:s1, :] * scale).T),
        "wk": np.ascontiguousarray(w_qkv[C + s0 : C + s1, :].T),
        "wv": np.ascontiguousarray(w_qkv[2 * C + s0 : 2 * C + s1, :].T),
        "wproj": np.ascontiguousarray(w_proj[:, s0:s1].T),
        "maskT": (mask[b].T.astype(np.float32) * MASK_NEG).astype(ml_dtypes.bfloat16),
        "ident": np.eye(128, dtype=ml_dtypes.bfloat16),
        "vones": np.ones((128, S // 128 * H_PER_CORE), dtype=np.float32),
    }


_NC_CACHE = {}


def get_nc():
    if "nc" not in _NC_CACHE:
        _NC_CACHE["nc"] = _build_kernel()
    return _NC_CACHE["nc"]


def _build_runner(nc):
    """Reusable jitted shard_map callable over the 8 cores (mirrors
    bass2jax.run_bass_via_pjrt but cacheable across calls)."""
    import jax
    from jax.experimental.shard_map import shard_map
    from jax.sharding import Mesh, PartitionSpec

    from concourse.bass2jax import (
        _bass_exec_p,
        install_neuronx_cc_hook,
        partition_id_tensor,
    )

    install_neuronx_cc_hook()
    partition_name = nc.partition_id_tensor.name if nc.partition_id_tensor else None
    in_names, out_names, out_avals, zero_outs = [], [], [], []
    for alloc in nc.m.functions[0].allocations:
        if not isinstance(alloc, mybir.MemoryLocationSet):
            continue
        name = alloc.memorylocations[0].name
        if alloc.kind == "ExternalInput":
            if name != partition_name:
                in_names.append(name)
        elif alloc.kind == "ExternalOutput":
            out_names.append(name)
            shape = tuple(alloc.tensor_shape)
            dtype = mybir.dt.np(alloc.dtype)
            out_avals.append(jax.core.ShapedArray(shape, dtype))
            zero_outs.append(np.zeros(shape, dtype))
    n_params = len(in_names)
    all_in_names = list(in_names) + list(out_names)
    if partition_name is not None:
        all_in_names.append(partition_name)

    def _body(*args):
        operands = list(args)
        if partition_name is not None:
            operands.append(partition_id_tensor())
        outs = _bass_exec_p.bind(
            *operands,
            out_avals=tuple(out_avals),
            in_names=tuple(all_in_names),
            out_names=tuple(out_names),
            lowering_input_output_aliases=(),
            sim_require_finite=True,
            sim_require_nnan=True,
            nc=nc,
        )
        return tuple(outs)

    n_cores = nc.num_devices
    devices = jax.devices()[:n_cores]
    mesh = Mesh(np.asarray(devices), ("core",))
    in_specs = (PartitionSpec("core"),) * (n_params + len(out_names))
    out_specs = (PartitionSpec("core"),) * len(out_names)
    fn = jax.jit(
        shard_map(
            _body, mesh=mesh, in_specs=in_specs, out_specs=out_specs, check_rep=False
        ),
        keep_unused=True,
    )
    return fn, in_names, out_names, zero_outs


_RUNNER_CACHE = {}


def get_runner(nc, in_maps):
    """Return (fn, dev_args) for repeated dispatch of `nc` with `in_maps`."""
    import jax
    from jax.sharding import Mesh, NamedSharding, PartitionSpec

    key = id(nc)
    if key not in _RUNNER_CACHE:
        _RUNNER_CACHE[key] = _build_runner(nc)
    fn, in_names, out_names, zero_outs = _RUNNER_CACHE[key]
    n_cores = nc.num_devices
    mesh = Mesh(np.asarray(jax.devices()[:n_cores]), ("core",))
    shard = NamedSharding(mesh, PartitionSpec("core"))
    concat_in = [
        np.concatenate([np.asarray(in_maps[c][n]) for c in range(n_cores)], axis=0)
        for n in in_names
    ]
    concat_zeros = [
        np.zeros((n_cores * z.shape[0], *z.shape[1:]), z.dtype) for z in zero_outs
    ]
    dev_args = [jax.device_put(a, shard) for a in concat_in + concat_zeros]
    return fn, dev_args


def run_cached(nc, in_maps):
    """Execute via the cached runner; returns per-core result dicts."""
    fn, dev_args = get_runner(nc, in_maps)
    out_arrs = fn(*dev_args)
    _, _, out_names, zero_outs = _RUNNER_CACHE[id(nc)]
    n_cores = nc.num_devices
    results = []
    for c in range(n_cores):
        results.append(
            {
                name: np.asarray(out_arrs[i]).reshape(
                    n_cores, *zero_outs[i].shape
                )[c]
                for i, name in enumerate(out_names)
            }
        )
    return results


def make_in_maps(x, mask, w_qkv, w_proj):
    return [_prep_core_inputs(x, mask, w_qkv, w_proj, c) for c in range(N_CORES)]


def combine(results, b_proj):
    outs = []
    for b in range(B):
        outs.append(results[2 * b]["y"] + results[2 * b + 1]["y"] + b_proj[None, :])
    return np.stack(outs).astype(np.float32)


def kernel(x, mask, w_qkv, w_proj, b_proj):
    x = np.asarray(x, dtype=np.float32)
    mask = np.asarray(mask)
    w_qkv = np.asarray(w_qkv, dtype=np.float32)
    w_proj = np.asarray(w_proj, dtype=np.float32)
    b_proj = np.asarray(b_proj, dtype=np.float32)

    nc = get_nc()
    in_maps = make_in_maps(x, mask, w_qkv, w_proj)
    try:
        results = run_cached(nc, in_maps)
    except Exception:
        results = run_bass_kernel_spmd(nc, in_maps, list(range(N_CORES))).results
    return combine(results, b_proj)


# revision 13
# speedup vs baseline: 1.0538x; 1.0538x over previous
"""TRN2 Bass kernel: masked multi-head attention block (B=4, S=2048, C=768, H=12).

Sharding: 8 cores = 4 batches x 2 head-groups (6 heads each).  Each core runs a
flash-attention-style Bass/Tile kernel over its (batch, head-group) shard:

  qT/kT: [384, S] feature-major projections from xT (q pre-scaled by hd^-0.5)
  v:     [S, 6*65] natural layout, a ones column appended per head
  scoresT[k, q] = k . q (contract hd=64, head pairs row-packed on the PE array)
  mask folded in as -1e5*maskT via identity-matmul on PE / in-place DVE add
  pT = exp(scoresT) on the scalar engine (psum -> sbuf)
  avT[65, 512] accumulated over key chunks; row 64 = softmax denominator (ones col)
  attn_outT = avT[0:64] * recip(denominator)  (partition-broadcast on gpsimd)
  y_partial = attn_outT.T @ w_projT slice  (row-parallel output projection)

Host-side: transposes/slices the weights per core, converts the mask to
pre-scaled bf16, sums the two per-batch partials, and adds b_proj.
"""

from contextlib import ExitStack

import numpy as np

import concourse.tile as tile
from concourse import bacc, mybir
from concourse.bass_utils import run_bass_kernel_spmd

F32 = mybir.dt.float32
F32R = mybir.dt.float32r
BF16 = mybir.dt.bfloat16

MASK_NEG = -100000.0
B, S, C, H = 4, 2048, 768, 12
HD = 64
H_PER_CORE = 6
D_CORE = H_PER_CORE * HD  # 384
QBLK = 512
GRP = 2
N_CORES = 8
MASK_DVE_FRAC = 1.0


def _r(ap):
    return ap.bitcast(F32R)


def _build_kernel(mask_dve_frac=MASK_DVE_FRAC):
    nc = bacc.Bacc(
        trn_type="TRN2", target_bir_lowering=False, debug=False, num_devices=N_CORES
    )
    KC = S // 128
    QB = S // QBLK
    NB = S // QBLK
    ST = S // 128
    groups = []
    for half in range(KC // 8):
        base = half * 8
        for g0, gs in ((0, 2), (2, 2), (4, 2), (6, 2)):
            groups.append((base + g0, gs))

    xT = nc.dram_tensor("xT", [C, S], F32R, kind="ExternalInput").ap()
    wq = nc.dram_tensor("wq", [C, D_CORE], F32R, kind="ExternalInput").ap()
    wk = nc.dram_tensor("wk", [C, D_CORE], F32R, kind="ExternalInput").ap()
    wv = nc.dram_tensor("wv", [C, D_CORE], F32R, kind="ExternalInput").ap()
    wproj = nc.dram_tensor("wproj", [D_CORE, C], F32R, kind="ExternalInput").ap()
    vones = nc.dram_tensor("vones", [128, S // 128 * H_PER_CORE], F32R, kind="ExternalInput").ap()
    maskT = nc.dram_tensor("maskT", [S, S], BF16, kind="ExternalInput").ap()
    ident = nc.dram_tensor("ident", [128, 128], BF16, kind="ExternalInput").ap()
    y = nc.dram_tensor("y", [S, C], F32, kind="ExternalOutput").ap()

    with tile.TileContext(nc) as tc, ExitStack() as ctx:
        consts = ctx.enter_context(tc.tile_pool(name="consts", bufs=1))
        qkv_pool = ctx.enter_context(tc.tile_pool(name="qkv", bufs=1))

        ident_sb = consts.tile([128, 128], BF16)
        nc.sync.dma_start(ident_sb[:], ident[:])
        wproj_sb = consts.tile([128, 3, C], F32R)
        nc.sync.dma_start(wproj_sb[:], wproj.rearrange("(t p) o -> p t o", p=128))

        qT_sb = qkv_pool.tile([128, 3, S], F32R)
        kT_sb = qkv_pool.tile([128, 3, S], F32R)
        vaug_sb = qkv_pool.tile([128, ST, H_PER_CORE * (HD + 1)], F32R)
        attn_sb = qkv_pool.tile([128, 3, S], F32R)

        # ones columns (softmax denominator) come from DRAM
        vaug_ones = vaug_sb.rearrange("p st (h u) -> p st h u", u=HD + 1)[:, :, :, HD]
        nc.sync.dma_start(
            vaug_ones, vones.rearrange("p (st h) -> p st h", h=H_PER_CORE)
        )

        # ---------------- phase 1: qkv projections ----------------
        with ExitStack() as p1:
            wpool = p1.enter_context(tc.tile_pool(name="w1", bufs=1))
            xpool = p1.enter_context(tc.tile_pool(name="x1", bufs=1))
            ps1 = p1.enter_context(tc.tile_pool(name="ps1", bufs=3, space="PSUM"))
            psv1 = p1.enter_context(tc.tile_pool(name="psv1", bufs=2, space="PSUM"))

            wq_sb = wpool.tile([128, 6, D_CORE], F32R)
            wk_sb = wpool.tile([128, 6, D_CORE], F32R)
            wv_sb = wpool.tile([128, 6, D_CORE], F32R)
            for w_ap, w_sb in ((wq, wq_sb), (wk, wk_sb), (wv, wv_sb)):
                nc.sync.dma_start(
                    w_sb[:], w_ap.rearrange("(t p) d -> p t d", p=128)
                )
            xT_sb = xpool.tile([128, 6, S], F32R)
            nc.sync.dma_start(xT_sb[:], xT.rearrange("(t p) s -> p t s", p=128))

            cp_i = 0
            for w_sb, dst in ((wq_sb, qT_sb), (wk_sb, kT_sb)):
                for m in range(3):
                    for nb in range(NB):
                        ps = ps1.tile([128, QBLK], F32, tag="psqk", name="psqk")
                        for k in range(6):
                            nc.tensor.matmul(
                                ps[:],
                                w_sb[:, k, m * 128 : (m + 1) * 128],
                                xT_sb[:, k, nb * QBLK : (nb + 1) * QBLK],
                                start=(k == 0),
                                stop=(k == 5),
                            )
                        dst_ap = dst[:, m, nb * QBLK : (nb + 1) * QBLK]
                        if cp_i % 2 == 0:
                            nc.vector.tensor_copy(dst_ap, ps[:])
                        else:
                            nc.scalar.copy(dst_ap, ps[:])
                        cp_i += 1

            for st in range(ST):
                psv = psv1.tile([128, D_CORE], F32, tag="psv", name="psv")
                for k in range(6):
                    nc.tensor.matmul(
                        psv[:],
                        xT_sb[:, k, st * 128 : (st + 1) * 128],
                        wv_sb[:, k, :],
                        start=(k == 0),
                        stop=(k == 5),
                    )
                for h in range(H_PER_CORE):
                    dst = vaug_sb[:, st, h * (HD + 1) : h * (HD + 1) + HD]
                    src = psv[:, h * HD : (h + 1) * HD]
                    if h % 2 == 0:
                        nc.vector.tensor_copy(dst, src)
                    else:
                        nc.scalar.copy(dst, src)

        # ---------------- phase 2: attention ----------------
        with ExitStack() as p2:
            mpool = p2.enter_context(tc.tile_pool(name="mask", bufs=4))
            ppool = p2.enter_context(tc.tile_pool(name="pT", bufs=5))
            dpool = p2.enter_context(tc.tile_pool(name="div", bufs=2))
            bpool = p2.enter_context(tc.tile_pool(name="bcast", bufs=2))
            apool = p2.enter_context(tc.tile_pool(name="avsb", bufs=1))
            ps_s = p2.enter_context(tc.tile_pool(name="ps_s", bufs=3, space="PSUM"))
            ps_av = p2.enter_context(tc.tile_pool(name="ps_av", bufs=2, space="PSUM"))

            mask_idx = 0
            maskT_r = maskT.rearrange("(kc p) q -> p kc q", p=128)
            mask_cache = {}

            def load_mask(qb_i):
                halves = []
                for half in range(KC // 8):
                    mh = mpool.tile([128, 8, QBLK], BF16, tag="mask", name="mask_h")
                    nc.sync.dma_start(
                        mh[:],
                        maskT_r[
                            :,
                            half * 8 : (half + 1) * 8,
                            qb_i * QBLK : (qb_i + 1) * QBLK,
                        ],
                    )
                    halves.append(mh)
                return halves

            mask_cache[0] = load_mask(0)
            for qb in range(QB):
                if qb + 1 < QB:
                    mask_cache[qb + 1] = load_mask(qb + 1)
                mask_halves = mask_cache.pop(qb)
                dstack = dpool.tile([H_PER_CORE, QBLK], F32, tag="dstack", name="dstack")
                recip = dpool.tile([H_PER_CORE, QBLK], F32, tag="recip", name="recip")
                av_all = apool.tile([HD + 1, H_PER_CORE, QBLK], F32, tag="av_all", name="av_all")
                av_keep = []

                for hp in range(3):
                    hA, hB = 2 * hp, 2 * hp + 1
                    av = [
                        ps_av.tile([HD + 1, QBLK], F32, tag="av", name=f"av{hp}a"),
                        ps_av.tile([HD + 1, QBLK], F32, tag="av", name=f"av{hp}b"),
                    ]
                    for (g0, gs) in groups:
                        mh = mask_halves[g0 // 8]
                        moff = g0 % 8
                        sc = [
                            ps_s.tile([128, GRP, QBLK], F32, tag="sc", name="scA"),
                            ps_s.tile([128, GRP, QBLK], F32, tag="sc", name="scB"),
                        ]
                        dve_heads = []
                        for i, h in ((0, hA), (1, hB)):
                            on_dve = (mask_idx % 10) < int(round(mask_dve_frac * 10))
                            mask_idx += 1
                            if on_dve:
                                dve_heads.append(i)
                            else:
                                for c in range(gs):
                                    nc.tensor.matmul(
                                        sc[i][:, c, :],
                                        ident_sb[:],
                                        mh[:, moff + c, :],
                                        start=True,
                                        stop=False,
                                    )
                            row0 = (h % 2) * HD
                            for c in range(gs):
                                kc = g0 + c
                                nc.tensor.matmul(
                                    sc[i][:, c, :],
                                    kT_sb[
                                        row0 : row0 + HD,
                                        h // 2,
                                        kc * 128 : (kc + 1) * 128,
                                    ],
                                    qT_sb[
                                        row0 : row0 + HD,
                                        h // 2,
                                        qb * QBLK : (qb + 1) * QBLK,
                                    ],
                                    start=on_dve,
                                    stop=True,
                                    tile_position=(row0, 0),
                                )
                        for i in dve_heads:
                            nc.vector.tensor_add(
                                sc[i][:, :gs, :],
                                sc[i][:, :gs, :],
                                mh[:, moff : moff + gs, :],
                            )

                        for i, h in ((0, hA), (1, hB)):
                            pT = ppool.tile([128, GRP, QBLK], F32R, tag="pT", name="pT")
                            nc.scalar.activation(
                                pT[:, :gs, :],
                                sc[i][:, :gs, :],
                                mybir.ActivationFunctionType.Exp,
                            )
                            for c in range(gs):
                                kc = g0 + c
                                nc.tensor.matmul(
                                    av[i][:],
                                    vaug_sb[:, kc, h * (HD + 1) : (h + 1) * (HD + 1)],
                                    _r(pT[:, c, :]),
                                    start=(kc == 0),
                                    stop=(kc == KC - 1),
                                )

                    for i, h in ((0, hA), (1, hB)):
                        nc.vector.tensor_copy(av_all[:, h, :], av[i][:])
                        av_keep.append(h)

                # gather all 6 denominator rows in one partition-shift DMA
                nc.gpsimd.dma_start(dstack[:], av_all[HD : HD + 1, :, :])
                nc.vector.reciprocal_approx_fast(recip[:], dstack[:])
                r6 = bpool.tile([1, H_PER_CORE, QBLK], F32, tag="r6", name="r6", bufs=1)
                nc.gpsimd.dma_start(r6[:], recip[:])
                tmp_all = bpool.tile(
                    [HD, 3, QBLK], F32R, tag="tmp_all", name="tmp_all", bufs=1
                )
                for h in av_keep:
                    bc = bpool.tile([HD, QBLK], F32, tag="bc", name="bc")
                    nc.gpsimd.partition_broadcast(bc[:], r6[:, h, :])
                    if h % 2 == 0:
                        dst = attn_sb[:HD, h // 2, qb * QBLK : (qb + 1) * QBLK]
                        nc.vector.tensor_mul(dst, av_all[:HD, h, :], bc[:])
                    else:
                        nc.vector.tensor_mul(
                            tmp_all[:, h // 2, :], av_all[:HD, h, :], bc[:]
                        )
                nc.gpsimd.dma_start(
                    attn_sb[HD:128, :, qb * QBLK : (qb + 1) * QBLK], tmp_all[:]
                )

        # ---------------- phase 3: output projection ----------------
        with ExitStack() as p3:
            ypool = p3.enter_context(tc.tile_pool(name="y", bufs=3))
            ps_y = p3.enter_context(tc.tile_pool(name="ps_y", bufs=4, space="PSUM"))
            y_r = y.rearrange("(st p) o -> st p o", p=128)
            for st in range(ST):
                y_sb = ypool.tile([128, C], F32, tag="ysb", name="y_sb")
                for nb2 in range(2):
                    ps = ps_y.tile([128, 384], F32, tag="psy", name="psy")
                    for k3 in range(3):
                        nc.tensor.matmul(
                            ps[:],
                            attn_sb[:, k3, st * 128 : (st + 1) * 128],
                            wproj_sb[:, k3, nb2 * 384 : (nb2 + 1) * 384],
                            start=(k3 == 0),
                            stop=(k3 == 2),
                        )
                    if nb2 == 0:
                        nc.vector.tensor_copy(y_sb[:, :384], ps[:])
                    else:
                        nc.scalar.copy(y_sb[:, 384:], ps[:])
                nc.sync.dma_start(y_r[st], y_sb[:])

    nc.compile()
    return nc


def _prep_core_inputs(x, mask, w_qkv, w_proj, core):
    import ml_dtypes

    b, g = core // 2, core % 2
    scale = HD ** -0.5
    s0, s1 = 384 * g, 384 * (g + 1)
    return {
        "xT": np.ascontiguousarray(x[b].T),
        "wq": np.ascontiguousarray((w_qkv[s0:s1, :] * scale).T),
        "wk": np.ascontiguousarray(w_qkv[C + s0 : C + s1, :].T),
        "wv": np.ascontiguousarray(w_qkv[2 * C + s0 : 2 * C + s1, :].T),
        "wproj": np.ascontiguousarray(w_proj[:, s0:s1].T),
        "maskT": (mask[b].T.astype(np.float32) * MASK_NEG).astype(ml_dtypes.bfloat16),
        "ident": np.eye(128, dtype=ml_dtypes.bfloat16),
        "vones": np.ones((128, S // 128 * H_PER_CORE), dtype=np.float32),
    }


_NC_CACHE = {}


def get_nc():
    if "nc" not in _NC_CACHE:
        _NC_CACHE["nc"] = _build_kernel()
    return _NC_CACHE["nc"]


def _build_runner(nc):
    """Reusable jitted shard_map callable over the 8 cores (mirrors
    bass2jax.run_bass_via_pjrt but cacheable across calls)."""
    import jax
    from jax.experimental.shard_map import shard_map
    from jax.sharding import Mesh, PartitionSpec

    from concourse.bass2jax import (
        _bass_exec_p,
        install_neuronx_cc_hook,
        partition_id_tensor,
    )

    install_neuronx_cc_hook()
    partition_name = nc.partition_id_tensor.name if nc.partition_id_tensor else None
    in_names, out_names, out_avals, zero_outs = [], [], [], []
    for alloc in nc.m.functions[0].allocations:
        if not isinstance(alloc, mybir.MemoryLocationSet):
            continue
        name = alloc.memorylocations[0].name
        if alloc.kind == "ExternalInput":
            if name != partition_name:
                in_names.append(name)
        elif alloc.kind == "ExternalOutput":
            out_names.append(name)
            shape = tuple(alloc.tensor_shape)
            dtype = mybir.dt.np(alloc.dtype)
            out_avals.append(jax.core.ShapedArray(shape, dtype))
            zero_outs.append(np.zeros(shape, dtype))
    n_params = len(in_names)
    all_in_names = list(in_names) + list(out_names)
    if partition_name is not None:
        all_in_names.append(partition_name)

    def _body(*args):
        operands = list(args)
        if partition_name is not None:
            operands.append(partition_id_tensor())
        outs = _bass_exec_p.bind(
            *operands,
            out_avals=tuple(out_avals),
            in_names=tuple(all_in_names),
            out_names=tuple(out_names),
            lowering_input_output_aliases=(),
            sim_require_finite=True,
            sim_require_nnan=True,
            nc=nc,
        )
        return tuple(outs)

    n_cores = nc.num_devices
    devices = jax.devices()[:n_cores]
    mesh = Mesh(np.asarray(devices), ("core",))
    in_specs = (PartitionSpec("core"),) * (n_params + len(out_names))
    out_specs = (PartitionSpec("core"),) * len(out_names)
    fn = jax.jit(
        shard_map(
            _body, mesh=mesh, in_specs=in_specs, out_specs=out_specs, check_rep=False
        ),
        keep_unused=True,
    )
    return fn, in_names, out_names, zero_outs


_RUNNER_CACHE = {}


def get_runner(nc, in_maps):
    """Return (fn, dev_args) for repeated dispatch of `nc` with `in_maps`."""
    import jax
    from jax.sharding import Mesh, NamedSharding, PartitionSpec

    key = id(nc)
    if key not in _RUNNER_CACHE:
        _RUNNER_CACHE[key] = _build_runner(nc)
    fn, in_names, out_names, zero_outs = _RUNNER_CACHE[key]
    n_cores = nc.num_devices
    mesh = Mesh(np.asarray(jax.devices()[:n_cores]), ("core",))
    shard = NamedSharding(mesh, PartitionSpec("core"))
    concat_in = [
        np.concatenate([np.asarray(in_maps[c][n]) for c in range(n_cores)], axis=0)
        for n in in_names
    ]
    concat_zeros = [
        np.zeros((n_cores * z.shape[0], *z.shape[1:]), z.dtype) for z in zero_outs
    ]
    dev_args = [jax.device_put(a, shard) for a in concat_in + concat_zeros]
    return fn, dev_args


def run_cached(nc, in_maps):
    """Execute via the cached runner; returns per-core result dicts."""
    fn, dev_args = get_runner(nc, in_maps)
    out_arrs = fn(*dev_args)
    _, _, out_names, zero_outs = _RUNNER_CACHE[id(nc)]
    n_cores = nc.num_devices
    results = []
    for c in range(n_cores):
        results.append(
            {
                name: np.asarray(out_arrs[i]).reshape(
                    n_cores, *zero_outs[i].shape
                )[c]
                for i, name in enumerate(out_names)
            }
        )
    return results


def make_in_maps(x, mask, w_qkv, w_proj):
    return [_prep_core_inputs(x, mask, w_qkv, w_proj, c) for c in range(N_CORES)]


def combine(results, b_proj):
    outs = []
    for b in range(B):
        outs.append(results[2 * b]["y"] + results[2 * b + 1]["y"] + b_proj[None, :])
    return np.stack(outs).astype(np.float32)


def kernel(x, mask, w_qkv, w_proj, b_proj):
    x = np.asarray(x, dtype=np.float32)
    mask = np.asarray(mask)
    w_qkv = np.asarray(w_qkv, dtype=np.float32)
    w_proj = np.asarray(w_proj, dtype=np.float32)
    b_proj = np.asarray(b_proj, dtype=np.float32)

    nc = get_nc()
    in_maps = make_in_maps(x, mask, w_qkv, w_proj)
    try:
        results = run_cached(nc, in_maps)
    except Exception:
        results = run_bass_kernel_spmd(nc, in_maps, list(range(N_CORES))).results
    return combine(results, b_proj)


# revision 15
# speedup vs baseline: 1.0565x; 1.0026x over previous
"""TRN2 Bass kernel: masked multi-head attention block (B=4, S=2048, C=768, H=12).

Sharding: 8 cores = 4 batches x 2 head-groups (6 heads each).  Each core runs a
flash-attention-style Bass/Tile kernel over its (batch, head-group) shard:

  qT/kT: [384, S] feature-major projections from xT (q pre-scaled by hd^-0.5)
  v:     [S, 6*65] natural layout, a ones column appended per head
  scoresT[k, q] = k . q (contract hd=64, head pairs row-packed on the PE array)
  mask folded in as -1e5*maskT via identity-matmul on PE / in-place DVE add
  pT = exp(scoresT) on the scalar engine (psum -> sbuf)
  avT[65, 512] accumulated over key chunks; row 64 = softmax denominator (ones col)
  attn_outT = avT[0:64] * recip(denominator)  (partition-broadcast on gpsimd)
  y_partial = attn_outT.T @ w_projT slice  (row-parallel output projection)

Host-side: transposes/slices the weights per core, converts the mask to
pre-scaled bf16, sums the two per-batch partials, and adds b_proj.
"""

from contextlib import ExitStack

import numpy as np

import concourse.tile as tile
from concourse import bacc, mybir
from concourse.bass_utils import run_bass_kernel_spmd

F32 = mybir.dt.float32
F32R = mybir.dt.float32r
BF16 = mybir.dt.bfloat16

MASK_NEG = -100000.0
B, S, C, H = 4, 2048, 768, 12
HD = 64
H_PER_CORE = 6
D_CORE = H_PER_CORE * HD  # 384
QBLK = 512
GRP = 2
N_CORES = 8
MASK_DVE_FRAC = 1.0


def _r(ap):
    return ap.bitcast(F32R)


def _build_kernel(mask_dve_frac=MASK_DVE_FRAC):
    nc = bacc.Bacc(
        trn_type="TRN2", target_bir_lowering=False, debug=False, num_devices=N_CORES
    )
    KC = S // 128
    QB = S // QBLK
    NB = S // QBLK
    ST = S // 128
    groups = []
    for half in range(KC // 8):
        base = half * 8
        for g0, gs in ((0, 2), (2, 2), (4, 2), (6, 2)):
            groups.append((base + g0, gs))

    xT = nc.dram_tensor("xT", [C, S], F32R, kind="ExternalInput").ap()
    wq = nc.dram_tensor("wq", [C, D_CORE], F32R, kind="ExternalInput").ap()
    wk = nc.dram_tensor("wk", [C, D_CORE], F32R, kind="ExternalInput").ap()
    wv = nc.dram_tensor("wv", [C, D_CORE], F32R, kind="ExternalInput").ap()
    wproj = nc.dram_tensor("wproj", [D_CORE, C], F32R, kind="ExternalInput").ap()
    vones = nc.dram_tensor("vones", [128, S // 128 * H_PER_CORE], F32R, kind="ExternalInput").ap()
    maskT = nc.dram_tensor("maskT", [S, S], BF16, kind="ExternalInput").ap()
    ident = nc.dram_tensor("ident", [128, 128], BF16, kind="ExternalInput").ap()
    y = nc.dram_tensor("y", [S, C], F32, kind="ExternalOutput").ap()

    with tile.TileContext(nc) as tc, ExitStack() as ctx:
        consts = ctx.enter_context(tc.tile_pool(name="consts", bufs=1))
        qkv_pool = ctx.enter_context(tc.tile_pool(name="qkv", bufs=1))

        ident_sb = consts.tile([128, 128], BF16)
        nc.sync.dma_start(ident_sb[:], ident[:])
        wproj_sb = consts.tile([128, 3, C], F32R)
        nc.sync.dma_start(wproj_sb[:], wproj.rearrange("(t p) o -> p t o", p=128))

        qT_sb = qkv_pool.tile([128, 3, S], F32R)
        kT_sb = qkv_pool.tile([128, 3, S], F32R)
        vaug_sb = qkv_pool.tile([128, ST, H_PER_CORE * (HD + 1)], F32R)
        attn_sb = qkv_pool.tile([128, 3, S], F32R)

        # ones columns (softmax denominator) come from DRAM
        vaug_ones = vaug_sb.rearrange("p st (h u) -> p st h u", u=HD + 1)[:, :, :, HD]
        nc.sync.dma_start(
            vaug_ones, vones.rearrange("p (st h) -> p st h", h=H_PER_CORE)
        )

        # ---------------- phase 1: qkv projections ----------------
        with ExitStack() as p1:
            wpool = p1.enter_context(tc.tile_pool(name="w1", bufs=1))
            xpool = p1.enter_context(tc.tile_pool(name="x1", bufs=1))
            ps1 = p1.enter_context(tc.tile_pool(name="ps1", bufs=3, space="PSUM"))
            psv1 = p1.enter_context(tc.tile_pool(name="psv1", bufs=2, space="PSUM"))

            wq_sb = wpool.tile([128, 6, D_CORE], F32R)
            wk_sb = wpool.tile([128, 6, D_CORE], F32R)
            wv_sb = wpool.tile([128, 6, D_CORE], F32R)
            for w_ap, w_sb in ((wq, wq_sb), (wk, wk_sb), (wv, wv_sb)):
                nc.sync.dma_start(
                    w_sb[:], w_ap.rearrange("(t p) d -> p t d", p=128)
                )
            xT_sb = xpool.tile([128, 6, S], F32R)
            nc.sync.dma_start(xT_sb[:], xT.rearrange("(t p) s -> p t s", p=128))

            cp_i = 0
            for w_sb, dst in ((wq_sb, qT_sb), (wk_sb, kT_sb)):
                for m in range(3):
                    for nb in range(NB):
                        ps = ps1.tile([128, QBLK], F32, tag="psqk", name="psqk")
                        for k in range(6):
                            nc.tensor.matmul(
                                ps[:],
                                w_sb[:, k, m * 128 : (m + 1) * 128],
                                xT_sb[:, k, nb * QBLK : (nb + 1) * QBLK],
                                start=(k == 0),
                                stop=(k == 5),
                            )
                        dst_ap = dst[:, m, nb * QBLK : (nb + 1) * QBLK]
                        if cp_i % 2 == 0:
                            nc.vector.tensor_copy(dst_ap, ps[:])
                        else:
                            nc.scalar.copy(dst_ap, ps[:])
                        cp_i += 1

            for st in range(ST):
                psv = psv1.tile([128, D_CORE], F32, tag="psv", name="psv")
                for k in range(6):
                    nc.tensor.matmul(
                        psv[:],
                        xT_sb[:, k, st * 128 : (st + 1) * 128],
                        wv_sb[:, k, :],
                        start=(k == 0),
                        stop=(k == 5),
                    )
                for h in range(H_PER_CORE):
                    dst = vaug_sb[:, st, h * (HD + 1) : h * (HD + 1) + HD]
                    src = psv[:, h * HD : (h + 1) * HD]
                    if h % 2 == 0:
                        nc.vector.tensor_copy(dst, src)
                    else:
                        nc.scalar.copy(dst, src)

        # ---------------- phase 2: attention ----------------
        with ExitStack() as p2:
            mpool = p2.enter_context(tc.tile_pool(name="mask", bufs=4))
            ppool = p2.enter_context(tc.tile_pool(name="pT", bufs=5))
            dpool = p2.enter_context(tc.tile_pool(name="div", bufs=2))
            bpool = p2.enter_context(tc.tile_pool(name="bcast", bufs=2))
            apool = p2.enter_context(tc.tile_pool(name="avsb", bufs=1))
            ps_s = p2.enter_context(tc.tile_pool(name="ps_s", bufs=3, space="PSUM"))
            ps_av = p2.enter_context(tc.tile_pool(name="ps_av", bufs=2, space="PSUM"))

            mask_idx = 0
            maskT_r = maskT.rearrange("(kc p) q -> p kc q", p=128)
            mask_cache = {}

            def load_mask(qb_i):
                halves = []
                for half in range(KC // 8):
                    mh = mpool.tile([128, 8, QBLK], BF16, tag="mask", name="mask_h")
                    nc.sync.dma_start(
                        mh[:],
                        maskT_r[
                            :,
                            half * 8 : (half + 1) * 8,
                            qb_i * QBLK : (qb_i + 1) * QBLK,
                        ],
                    )
                    halves.append(mh)
                return halves

            mask_cache[0] = load_mask(0)
            for qb in range(QB):
                if qb + 1 < QB:
                    mask_cache[qb + 1] = load_mask(qb + 1)
                mask_halves = mask_cache.pop(qb)
                dstack = dpool.tile([H_PER_CORE, QBLK], F32, tag="dstack", name="dstack")
                recip = dpool.tile([H_PER_CORE, QBLK], F32, tag="recip", name="recip")
                av_all = apool.tile([HD + 1, H_PER_CORE, QBLK], F32, tag="av_all", name="av_all")
                av_keep = []

                for hp in range(3):
                    hA, hB = 2 * hp, 2 * hp + 1
                    av = [
                        ps_av.tile([HD + 1, QBLK], F32, tag="av", name=f"av{hp}a"),
                        ps_av.tile([HD + 1, QBLK], F32, tag="av", name=f"av{hp}b"),
                    ]
                    for (g0, gs) in groups:
                        mh = mask_halves[g0 // 8]
                        moff = g0 % 8
                        sc = [
                            ps_s.tile([128, GRP, QBLK], F32, tag="sc", name="scA"),
                            ps_s.tile([128, GRP, QBLK], F32, tag="sc", name="scB"),
                        ]
                        dve_heads = []
                        for i, h in ((0, hA), (1, hB)):
                            on_dve = (mask_idx % 10) < int(round(mask_dve_frac * 10))
                            mask_idx += 1
                            if on_dve:
                                dve_heads.append(i)
                            else:
                                for c in range(gs):
                                    nc.tensor.matmul(
                                        sc[i][:, c, :],
                                        ident_sb[:],
                                        mh[:, moff + c, :],
                                        start=True,
                                        stop=False,
                                    )
                            row0 = (h % 2) * HD
                            for c in range(gs):
                                kc = g0 + c
                                nc.tensor.matmul(
                                    sc[i][:, c, :],
                                    kT_sb[
                                        row0 : row0 + HD,
                                        h // 2,
                                        kc * 128 : (kc + 1) * 128,
                                    ],
                                    qT_sb[
                                        row0 : row0 + HD,
                                        h // 2,
                                        qb * QBLK : (qb + 1) * QBLK,
                                    ],
                                    start=on_dve,
                                    stop=True,
                                    tile_position=(row0, 0),
                                )
                        for i in dve_heads:
                            nc.vector.tensor_add(
                                sc[i][:, :gs, :],
                                sc[i][:, :gs, :],
                                mh[:, moff : moff + gs, :],
                            )

                        for i, h in ((0, hA), (1, hB)):
                            pT = ppool.tile([128, GRP, QBLK], F32R, tag="pT", name="pT")
                            nc.scalar.activation(
                                pT[:, :gs, :],
                                sc[i][:, :gs, :],
                                mybir.ActivationFunctionType.Exp,
                            )
                            for c in range(gs):
                                kc = g0 + c
                                nc.tensor.matmul(
                                    av[i][:],
                                    vaug_sb[:, kc, h * (HD + 1) : (h + 1) * (HD + 1)],
                                    _r(pT[:, c, :]),
                                    start=(kc == 0),
                                    stop=(kc == KC - 1),
                                )

                    for i, h in ((0, hA), (1, hB)):
                        nc.vector.tensor_copy(av_all[:, h, :], av[i][:])
                        av_keep.append(h)

                # gather all 6 denominator rows in one partition-shift DMA
                nc.gpsimd.dma_start(dstack[:], av_all[HD : HD + 1, :, :])
                nc.vector.reciprocal_approx_fast(recip[:], dstack[:])
                r6 = bpool.tile([1, H_PER_CORE, QBLK], F32, tag="r6", name="r6", bufs=1)
                nc.gpsimd.dma_start(r6[:], recip[:])
                tmp_all = bpool.tile(
                    [HD, 3, QBLK], F32R, tag="tmp_all", name="tmp_all", bufs=1
                )
                for h in av_keep:
                    bc = bpool.tile([HD, QBLK], F32, tag="bc", name="bc")
                    nc.gpsimd.partition_broadcast(bc[:], r6[:, h, :])
                    if h % 2 == 0:
                        dst = attn_sb[:HD, h // 2, qb * QBLK : (qb + 1) * QBLK]
                        nc.vector.tensor_mul(dst, av_all[:HD, h, :], bc[:])
                    else:
                        nc.vector.tensor_mul(
                            tmp_all[:, h // 2, :], av_all[:HD, h, :], bc[:]
                        )
                nc.gpsimd.dma_start(
                    attn_sb[HD:128, :, qb * QBLK : (qb + 1) * QBLK], tmp_all[:]
                )

        # ---------------- phase 3: output projection ----------------
        with ExitStack() as p3:
            ypool = p3.enter_context(tc.tile_pool(name="y", bufs=3))
            ps_y = p3.enter_context(tc.tile_pool(name="ps_y", bufs=4, space="PSUM"))
            y_r = y.rearrange("(st p) o -> st p o", p=128)
            for st in range(ST):
                y_sb = ypool.tile([128, C], F32, tag="ysb", name="y_sb")
                for nb2 in range(2):
                    ps = ps_y.tile([128, 384], F32, tag="psy", name="psy")
                    for k3 in range(3):
                        nc.tensor.matmul(
                            ps[:],
                            attn_sb[:, k3, st * 128 : (st + 1) * 128],
                            wproj_sb[:, k3, nb2 * 384 : (nb2 + 1) * 384],
                            start=(k3 == 0),
                            stop=(k3 == 2),
                        )
                    if nb2 == 0:
                        nc.vector.tensor_copy(y_sb[:, :384], ps[:])
                    else:
                        nc.scalar.copy(y_sb[:, 384:], ps[:])
                nc.sync.dma_start(y_r[st], y_sb[:])

    nc.compile()
    return nc


def _prep_core_inputs(x, mask, w_qkv, w_proj, core):
    import ml_dtypes

    b, g = core // 2, core % 2
    scale = HD ** -0.5
    s0, s1 = 384 * g, 384 * (g + 1)
    return {
        "xT": np.ascontiguousarray(x[b].T),
        "wq": np.ascontiguousarray((w_qkv[s0:s1, :] * scale).T),
        "wk": np.ascontiguousarray(w_qkv[C + s0 : C + s1, :].T),
        "wv": np.ascontiguousarray(w_qkv[2 * C + s0 : 2 * C + s1, :].T),
        "wproj": np.ascontiguousarray(w_proj[:, s0:s1].T),
        "maskT": np.array([0.0, MASK_NEG], dtype=ml_dtypes.bfloat16)[mask[b].T],
        "ident": np.eye(128, dtype=ml_dtypes.bfloat16),
        "vones": np.ones((128, S // 128 * H_PER_CORE), dtype=np.float32),
    }


_NC_CACHE = {}


def get_nc():
    if "nc" not in _NC_CACHE:
        _NC_CACHE["nc"] = _build_kernel()
    return _NC_CACHE["nc"]


def _build_runner(nc):
    """Reusable jitted shard_map callable over the 8 cores (mirrors
    bass2jax.run_bass_via_pjrt but cacheable across calls)."""
    import jax
    from jax.experimental.shard_map import shard_map
    from jax.sharding import Mesh, PartitionSpec

    from concourse.bass2jax import (
        _bass_exec_p,
        install_neuronx_cc_hook,
        partition_id_tensor,
    )

    install_neuronx_cc_hook()
    partition_name = nc.partition_id_tensor.name if nc.partition_id_tensor else None
    in_names, out_names, out_avals, zero_outs = [], [], [], []
    for alloc in nc.m.functions[0].allocations:
        if not isinstance(alloc, mybir.MemoryLocationSet):
            continue
        name = alloc.memorylocations[0].name
        if alloc.kind == "ExternalInput":
            if name != partition_name:
                in_names.append(name)
        elif alloc.kind == "ExternalOutput":
            out_names.append(name)
            shape = tuple(alloc.tensor_shape)
            dtype = mybir.dt.np(alloc.dtype)
            out_avals.append(jax.core.ShapedArray(shape, dtype))
            zero_outs.append(np.zeros(shape, dtype))
    n_params = len(in_names)
    all_in_names = list(in_names) + list(out_names)
    if partition_name is not None:
        all_in_names.append(partition_name)

    def _body(*args):
        operands = list(args)
        if partition_name is not None:
            operands.append(partition_id_tensor())
        outs = _bass_exec_p.bind(
            *operands,
            out_avals=tuple(out_avals),
            in_names=tuple(all_in_names),
            out_names=tuple(out_names),
            lowering_input_output_aliases=(),
            sim_require_finite=True,
            sim_require_nnan=True,
            nc=nc,
        )
        return tuple(outs)

    n_cores = nc.num_devices
    devices = jax.devices()[:n_cores]
    mesh = Mesh(np.asarray(devices), ("core",))
    in_specs = (PartitionSpec("core"),) * (n_params + len(out_names))
    out_specs = (PartitionSpec("core"),) * len(out_names)
    fn = jax.jit(
        shard_map(
            _body, mesh=mesh, in_specs=in_specs, out_specs=out_specs, check_rep=False
        ),
        keep_unused=True,
    )
    return fn, in_names, out_names, zero_outs


_RUNNER_CACHE = {}


def get_runner(nc, in_maps):
    """Return (fn, dev_args) for repeated dispatch of `nc` with `in_maps`."""
    import jax
    from jax.sharding import Mesh, NamedSharding, PartitionSpec

    key = id(nc)
    if key not in _RUNNER_CACHE:
        _RUNNER_CACHE[key] = _build_runner(nc)
    fn, in_names, out_names, zero_outs = _RUNNER_CACHE[key]
    n_cores = nc.num_devices
    mesh = Mesh(np.asarray(jax.devices()[:n_cores]), ("core",))
    shard = NamedSharding(mesh, PartitionSpec("core"))
    concat_in = [
        np.concatenate([np.asarray(in_maps[c][n]) for c in range(n_cores)], axis=0)
        for n in in_names
    ]
    dev_in = [jax.device_put(a, shard) for a in concat_in]
    zkey = ("zeros", key)
    if zkey not in _RUNNER_CACHE:
        concat_zeros = [
            np.zeros((n_cores * z.shape[0], *z.shape[1:]), z.dtype) for z in zero_outs
        ]
        _RUNNER_CACHE[zkey] = [jax.device_put(a, shard) for a in concat_zeros]
    return fn, dev_in + _RUNNER_CACHE[zkey]


def run_cached(nc, in_maps):
    """Execute via the cached runner; returns per-core result dicts."""
    fn, dev_args = get_runner(nc, in_maps)
    out_arrs = fn(*dev_args)
    _, _, out_names, zero_outs = _RUNNER_CACHE[id(nc)]
    n_cores = nc.num_devices
    fetched = [
        np.asarray(a).reshape(n_cores, *zero_outs[i].shape)
        for i, a in enumerate(out_arrs)
    ]
    return [
        {name: fetched[i][c] for i, name in enumerate(out_names)}
        for c in range(n_cores)
    ]


def make_in_maps(x, mask, w_qkv, w_proj):
    return [_prep_core_inputs(x, mask, w_qkv, w_proj, c) for c in range(N_CORES)]


def combine(results, b_proj):
    outs = []
    for b in range(B):
        outs.append(results[2 * b]["y"] + results[2 * b + 1]["y"] + b_proj[None, :])
    return np.stack(outs).astype(np.float32)


def kernel(x, mask, w_qkv, w_proj, b_proj):
    x = np.asarray(x, dtype=np.float32)
    mask = np.asarray(mask)
    w_qkv = np.asarray(w_qkv, dtype=np.float32)
    w_proj = np.asarray(w_proj, dtype=np.float32)
    b_proj = np.asarray(b_proj, dtype=np.float32)

    nc = get_nc()
    in_maps = make_in_maps(x, mask, w_qkv, w_proj)
    try:
        results = run_cached(nc, in_maps)
    except Exception:
        results = run_bass_kernel_spmd(nc, in_maps, list(range(N_CORES))).results
    return combine(results, b_proj)
